# revision 12
# baseline (speedup 1.0000x reference)
"""DeepSetPred Trainium2 kernel: token encoder MLP + segment-sum + predictor
MLP on 8 NeuronCores, zero collectives.

Sharding: the host cuts the (sorted-by-segment) token axis at segment
boundaries, so every segment belongs to exactly one core. Each shard is
padded to a common length with tokens whose one-hot selector row is all
zero. Each core computes the complete segment sums for its own contiguous
range of <=32 segments, runs the predictor on just those rows, and writes
its private slice of the output; the host concatenates.

Key algebraic restructure vs the straightforward version: the third
encoder linear (W3) commutes with the segment sum, so the per-token path
is only L1 -> tanh -> L2 -> (+b2) -> tanh -> segment-sum; W3 is applied
once to the [SLOTS, H] pooled matrix in fp32 at the tail. L2 emits
tokens-on-partitions ([tok, j]) so the segment-sum matmul consumes h2
directly (sel one-hot as the stationary operand); b2 (which then lands on
the free dim, out of reach of the ACT bias port) is added by the DVE from
a partition-replicated copy. All per-token matmuls are full K=128/M=128
fp16 with N=512 moving, which is the PE stream roofline for this network.
"""

import numpy as np

import concourse.mybir as mybir
import concourse.tile as tile
from concourse import bacc
from concourse import bass_utils
from concourse.masks import make_identity

# Problem shapes (hardcoded per contract).
T, E, H, C, O = 131072, 256, 512, 256, 32
S = 128            # num segments
N_CORES = 8
TOK = 512          # tokens per chunk
MIN_SLOTS = 32     # baseline segments-per-core capacity
F32 = mybir.dt.float32
F32R = mybir.dt.float32r
F16 = mybir.dt.float16

_CACHE = {}


def _mm(nc, out, lhsT, rhs, start, stop, skip=True):
    nc.tensor.matmul(out, lhsT, rhs,
                     start=start, stop=stop, skip_group_check=skip)


def _build_nc(t_sh, SLOTS):
    assert t_sh % 128 == 0

    nc = bacc.Bacc("TRN2", target_bir_lowering=False, debug=False,
                   num_devices=N_CORES)

    xt_d = nc.dram_tensor("xt", [E, t_sh], F16, kind="ExternalInput")
    sel_d = nc.dram_tensor("sel", [t_sh, SLOTS], F16, kind="ExternalInput")
    cnt_d = nc.dram_tensor("cnt", [1, SLOTS], F32, kind="ExternalInput")
    w1_d = nc.dram_tensor("w1", [E, H], F16, kind="ExternalInput")
    w2_d = nc.dram_tensor("w2", [H, H], F16, kind="ExternalInput")
    w3_d = nc.dram_tensor("w3", [H, C], F32R, kind="ExternalInput")
    brow_d = nc.dram_tensor("brow", [128, 12], F32, kind="ExternalInput")
    b2r_d = nc.dram_tensor("b2r", [128, H], F32, kind="ExternalInput")
    b3_d = nc.dram_tensor("b3", [1, C], F32, kind="ExternalInput")
    p1_d = nc.dram_tensor("p1", [C, H], F32R, kind="ExternalInput")
    p2_d = nc.dram_tensor("p2", [H, H], F32R, kind="ExternalInput")
    p3_d = nc.dram_tensor("p3", [H, O], F32R, kind="ExternalInput")
    pb3_d = nc.dram_tensor("pb3", [1, O], F32, kind="ExternalInput")
    out_d = nc.dram_tensor("pred", [SLOTS, O], F32, kind="ExternalOutput")

    EC = E // 128   # 2
    HC = H // 128   # 4
    CC = C // 128   # 2
    TT = TOK // 128  # 4 token sub-tiles per chunk

    with tile.TileContext(nc) as tc:
        with tc.tile_pool(name="wts", bufs=1) as wp, \
             tc.tile_pool(name="xt", bufs=3) as xtp, \
             tc.tile_pool(name="sel", bufs=4) as selp, \
             tc.tile_pool(name="h1", bufs=2) as h1p, \
             tc.tile_pool(name="z2", bufs=2) as z2p, \
             tc.tile_pool(name="h2", bufs=3) as h2p, \
             tc.tile_pool(name="small", bufs=1) as smp, \
             tc.tile_pool(name="ps", bufs=1, space="PSUM") as psp, \
             tc.tile_pool(name="psacc", bufs=1, space="PSUM") as psa:

            # warm the ACT tanh table before the scalar queue fills with DMAs
            warm_sb = smp.tile([1, 1], F32, tag="warm", name="warm")
            nc.gpsimd.memset(warm_sb[:], 0.0)
            warm_o = smp.tile([1, 1], F32, tag="warmo", name="warmo")
            nc.scalar.activation(warm_o[:], warm_sb[:],
                                 mybir.ActivationFunctionType.Tanh)

            # ---- encoder weights, needed immediately. w1/w2 are split
            # half-and-half across the scalar and vector rings so neither
            # transfer serializes behind the other; the biases ride the
            # otherwise-idle gpsimd ring as one small contiguous row. ----
            w1_ap = w1_d.ap().rearrange("(e p) (h q) -> p e h q",
                                        p=128, q=128)
            w1_t = wp.tile([128, EC, HC, 128], F16, tag="w1", name="w1t")
            nc.scalar.dma_start(w1_t[:], w1_ap)
            w1_sb = [[w1_t[:, e, h, :] for h in range(HC)] for e in range(EC)]
            w2_ap = w2_d.ap().rearrange("(k p) j -> p k j", p=128)
            w2_t = wp.tile([128, HC, H], F16, tag="w2", name="w2t")
            nc.scalar.dma_start(w2_t[:, 0:2], w2_ap[:, 0:2])
            w2_sb = [w2_t[:, k, :] for k in range(HC)]
            brow_sb = smp.tile([128, 12], F32, tag="brow", name="brow")
            nc.gpsimd.dma_start(brow_sb[:], brow_d.ap())
            b1_sb = brow_sb[:, 0:HC]
            pb1_sb = brow_sb[:, HC:2 * HC]
            pb2_sb = brow_sb[:, 2 * HC:3 * HC]
            b2r_sb = smp.tile([128, H], F32, tag="b2r", name="b2r")
            nc.gpsimd.dma_start(b2r_sb[:], b2r_d.ap())
            nc.gpsimd.dma_start(w2_t[:, 2:4], w2_ap[:, 2:4])

            # ---- persistent segment-sum accumulator encH[slot, j] ----
            encH_ps = psa.tile([SLOTS, H], F32, tag="encacc", name="encacc")

            # ---- main token loop: software-pipelined three-stage skew.
            # Emit order per step i: A(i)=DMA+L1, B(i-1)=L2+bias+tanh,
            # C(i-2)=segment matmul. The 2-chunk delay on C gives the
            # DVE bias-add + ACT tanh a full PE step of slack, so the PE
            # never waits on h2. ----
            n_full = t_sh // TOK
            tail = t_sh - n_full * TOK
            chunks = [(i * TOK, TOK) for i in range(n_full)]
            if tail:
                chunks.append((n_full * TOK, tail))
            assert len(chunks) >= 3  # tail weights are emitted at ci == 2

            def stage_a(base, tok):
                xt_t = xtp.tile([128, EC, tok], F16, tag="xt", name="xt",
                                padded_shape=[128, EC, TOK])
                nc.sync.dma_start(
                    xt_t[:],
                    xt_d.ap()[:, base:base + tok]
                        .rearrange("(e p) t -> p e t", p=128))
                sel_t = selp.tile([128, tok // 128, SLOTS], F16, tag="sel",
                                  name="sel", padded_shape=[128, TT, SLOTS])
                nc.sync.dma_start(
                    sel_t[:],
                    sel_d.ap()[base:base + tok, :]
                         .rearrange("(q p) s -> p q s", p=128))
                h1_t = h1p.tile([128, HC, tok], F16, tag="h1", name="h1",
                                padded_shape=[128, HC, TOK])
                for h in range(HC):
                    ps1 = psp.tile([128, tok], F32, tag="mm", name="mm",
                                   bufs=5, padded_shape=[128, TOK])
                    for e in range(EC):
                        _mm(nc, ps1[:], w1_sb[e][h], xt_t[:, e, :],
                            start=(e == 0), stop=(e == EC - 1))
                    nc.scalar.activation(h1_t[:, h, :], ps1[:],
                                         mybir.ActivationFunctionType.Tanh,
                                         bias=b1_sb[:, h:h + 1])
                return sel_t, h1_t

            def stage_b(sel_t, h1_t, tok):
                tt = tok // 128
                z2_t = z2p.tile([128, tt, H], F32, tag="z2", name="z2",
                                padded_shape=[128, TT, H])
                h2_t = h2p.tile([128, tt, H], F16, tag="h2", name="h2",
                                padded_shape=[128, TT, H])
                for t0 in range(0, tt, 2):
                    th = min(2, tt - t0)
                    psz = psp.tile([128, th, H], F32, tag="z", name="z",
                                   bufs=1, padded_shape=[128, 2, H])
                    for t in range(th):
                        for k in range(HC):
                            _mm(nc, psz[:, t, :],
                                h1_t[:, k, (t0 + t) * 128:(t0 + t + 1) * 128],
                                w2_sb[k],
                                start=(k == 0), stop=(k == HC - 1))
                    for t in range(th):
                        nc.vector.tensor_add(z2_t[:, t0 + t, :],
                                             psz[:, t, :], b2r_sb[:])
                    nc.scalar.activation(h2_t[:, t0:t0 + th, :],
                                         z2_t[:, t0:t0 + th, :],
                                         mybir.ActivationFunctionType.Tanh)
                return sel_t, h2_t

            seg_state = {"opened": False}

            def stage_c(sel_t, h2_t, tok, is_last):
                tt = tok // 128
                for t in range(tt):
                    first = not seg_state["opened"]
                    seg_state["opened"] = True
                    last = is_last and (t == tt - 1)
                    _mm(nc, encH_ps[:], sel_t[:, t, :], h2_t[:, t, :],
                        start=first, stop=last)

            pend_b = []
            pend_c = []
            for ci, (base, tok) in enumerate(chunks):
                if pend_b:
                    ab, tokb = pend_b.pop(0)
                    pend_c.append((stage_b(*ab, tokb), tokb))
                pend_b.append((stage_a(base, tok), tok))
                if ci == 2:
                    # tail weights: every DMA destination first gets a
                    # junk element copied from h1 (RAW on h1, then WAW for
                    # the DMA), so the scheduler cannot hoist these
                    # transfers into the startup HBM burst
                    gate = pend_b[-1][0][1]  # h1 tile of chunk 2
                    gsrc = gate[0:1, 0, 0:1]

                    def gated_dma(tile_ap, gate_dst, dram_ap):
                        nc.gpsimd.tensor_copy(gate_dst, gsrc)
                        nc.gpsimd.dma_start(tile_ap, dram_ap)

                    w3_t = wp.tile([128, HC, C], F32R, tag="w3", name="w3t")
                    gated_dma(w3_t[:], w3_t[0:1, 0, 0:1],
                              w3_d.ap().rearrange("(k p) c -> p k c", p=128))
                    p1_t = wp.tile([128, CC, HC, 128], F32R, tag="p1",
                                   name="p1t")
                    gated_dma(p1_t[:], p1_t[0:1, 0, 0, 0:1],
                              p1_d.ap().rearrange("(c p) (h q) -> p c h q",
                                                  p=128, q=128))
                    p2_t = wp.tile([128, HC, HC, 128], F32R, tag="p2",
                                   name="p2t")
                    gated_dma(p2_t[:], p2_t[0:1, 0, 0, 0:1],
                              p2_d.ap().rearrange("(k p) (h q) -> p k h q",
                                                  p=128, q=128))
                    p3_t = wp.tile([128, HC, O], F32R, tag="p3", name="p3t")
                    gated_dma(p3_t[:], p3_t[0:1, 0, 0:1],
                              p3_d.ap().rearrange("(k p) o -> p k o", p=128))
                    b3row = smp.tile([1, C], F32, tag="b3row", name="b3row")
                    gated_dma(b3row[:], b3row[0:1, 0:1], b3_d.ap())
                    pb3row = smp.tile([1, O], F32, tag="pb3row", name="pb3row")
                    gated_dma(pb3row[:], pb3row[0:1, 0:1], pb3_d.ap())
                    cntrow = smp.tile([1, SLOTS], F32, tag="cntrow",
                                      name="cntrow")
                    gated_dma(cntrow[:], cntrow[0:1, 0:1], cnt_d.ap())
                    ones1 = smp.tile([1, SLOTS], F32, tag="ones1",
                                     name="ones1")
                    nc.gpsimd.memset(ones1[:], 1.0)
                    ident = smp.tile([SLOTS, SLOTS], F32, tag="ident",
                                     name="ident")
                    make_identity(nc, ident[:])
                    w3_sb = [w3_t[:, k, :] for k in range(HC)]
                    p1_sb = [[p1_t[:, c, h, :] for h in range(HC)]
                             for c in range(CC)]
                    p2_sb = [[p2_t[:, k, h, :] for h in range(HC)]
                             for k in range(HC)]
                    p3_sb = [p3_t[:, k, :] for k in range(HC)]
                if len(pend_c) > 1:
                    cc_, tokc = pend_c.pop(0)
                    stage_c(*cc_, tokc, is_last=False)
            while pend_b:
                ab, tokb = pend_b.pop(0)
                pend_c.append((stage_b(*ab, tokb), tokb))
            while pend_c:
                cc_, tokc = pend_c.pop(0)
                stage_c(*cc_, tokc, is_last=(len(pend_c) == 0))

            # ---- tail: pooled W3 + counts*b3, then the predictor on this
            # core's own <=SLOTS segment rows ----
            encH_sb = smp.tile([SLOTS, H], F32, tag="encHsb", name="encHsb")
            nc.vector.tensor_copy(encH_sb[:], encH_ps[:])
            encHT = smp.tile([128, HC, SLOTS], F32R, tag="encHT",
                             name="encHT")
            for k in range(HC):
                pst = psp.tile([128, SLOTS], F32, tag="mm", name="tr", bufs=5)
                nc.tensor.transpose(pst[:],
                                    encH_sb[:, k * 128:(k + 1) * 128],
                                    ident[:])
                nc.vector.tensor_copy(encHT[:, k, :], pst[:])

            # encT[c, s] = sum_k W3[k,c-slice].T @ encHT[k] + b3.T @ cnt
            encT_sb = smp.tile([128, CC, SLOTS], F32R, tag="encT",
                               name="encT")
            for c in range(CC):
                pc = psp.tile([128, SLOTS], F32, tag="mm", name="pc", bufs=5)
                nc.tensor.matmul(pc[:], b3row[:, c * 128:(c + 1) * 128],
                                 cntrow[:], start=True, stop=False,
                                 skip_group_check=True)
                for k in range(HC):
                    _mm(nc, pc[:], w3_sb[k][:, c * 128:(c + 1) * 128],
                        encHT[:, k, :], start=False, stop=(k == HC - 1))
                nc.vector.tensor_copy(encT_sb[:, c, :], pc[:])

            q1_sb = smp.tile([128, HC, SLOTS], F32R, tag="q1", name="q1")
            for h in range(HC):
                pp1 = psp.tile([128, SLOTS], F32, tag="mm", name="pp1", bufs=5)
                for c in range(CC):
                    _mm(nc, pp1[:], p1_sb[c][h], encT_sb[:, c, :],
                        start=(c == 0), stop=(c == CC - 1))
                nc.scalar.activation(q1_sb[:, h, :], pp1[:],
                                     mybir.ActivationFunctionType.Tanh,
                                     bias=pb1_sb[:, h:h + 1])
            q2_sb = smp.tile([128, HC, SLOTS], F32R, tag="q2", name="q2")
            for h in range(HC):
                pp2 = psp.tile([128, SLOTS], F32, tag="mm", name="pp2", bufs=5)
                for k in range(HC):
                    _mm(nc, pp2[:], p2_sb[k][h], q1_sb[:, k, :],
                        start=(k == 0), stop=(k == HC - 1))
                nc.scalar.activation(q2_sb[:, h, :], pp2[:],
                                     mybir.ActivationFunctionType.Tanh,
                                     bias=pb2_sb[:, h:h + 1])

            # final: pred[slot, o] = q2.T @ P3 + pb3
            ppo = psp.tile([SLOTS, O], F32, tag="mm", name="ppo", bufs=5)
            nc.tensor.matmul(ppo[:], ones1[:], pb3row[:],
                             start=True, stop=False, skip_group_check=True)
            for k in range(HC):
                _mm(nc, ppo[:], q2_sb[:, k, :], p3_sb[k],
                    start=False, stop=(k == HC - 1))
            pred_sb = smp.tile([SLOTS, O], F32, tag="pred", name="predsb")
            nc.vector.tensor_copy(pred_sb[:], ppo[:])
            nc.sync.dma_start(out_d.ap(), pred_sb[:])

    nc.compile()
    return nc


def kernel(words, seg_ids, W1, b1, W2, b2, W3, b3,
           P1, pb1, P2, pb2, P3, pb3, batch_size, alpha_iter, **_):
    words = np.asarray(words, dtype=np.float32)
    seg_ids = np.asarray(seg_ids).astype(np.int64)
    assert words.shape == (T, E), words.shape
    bs, ai = int(batch_size), int(alpha_iter)

    # --- host-side index prep: cut the sorted token axis at segment
    # boundaries so each core owns whole segments ---
    counts = np.bincount(seg_ids, minlength=S)[:S]
    starts = np.concatenate([[0], np.cumsum(counts)])   # [S+1]
    cuts = [0]
    for c in range(1, N_CORES):
        tgt = c * T // N_CORES
        j = int(np.searchsorted(starts, tgt, side="left"))
        if j > 0 and tgt - starts[j - 1] < starts[j] - tgt:
            j -= 1
        cuts.append(int(starts[j]))
    cuts.append(T)
    lens = np.diff(cuts)
    t_sh = int(np.ceil(lens.max() / 128) * 128)

    # contiguous segment range owned by each core (covers all of [0, S));
    # empty shards inherit the following shard's start so ranges stay
    # monotone and collectively exhaustive
    seg_lo = [0] * N_CORES
    for c in range(N_CORES - 1, 0, -1):
        if lens[c] > 0:
            seg_lo[c] = int(seg_ids[cuts[c]])
        else:
            seg_lo[c] = S if c == N_CORES - 1 else seg_lo[c + 1]
    seg_hi = seg_lo[1:] + [S]
    slots_needed = max(seg_hi[c] - seg_lo[c] for c in range(N_CORES))
    SLOTS = min(128, max(MIN_SLOTS, ((slots_needed + 31) // 32) * 32))
    assert slots_needed <= SLOTS, (seg_lo, seg_hi)
    assert bs * ai == S

    xt = np.ascontiguousarray(words.T.astype(np.float16))    # [E, T] fp16

    key = ("nc", t_sh, SLOTS)
    if key not in _CACHE:
        _CACHE[key] = _build_nc(t_sh, SLOTS)
    nc = _CACHE[key]

    b2rep = np.ascontiguousarray(
        np.broadcast_to(np.asarray(b2, dtype=np.float32), (128, H)))
    # [128, 12]: cols 0-3 b1, 4-7 pb1, 8-11 pb2 (per-partition bias lanes)
    brow = np.concatenate([
        np.asarray(v, dtype=np.float32).reshape(H // 128, 128).T
        for v in (b1, pb1, pb2)], axis=1)
    brow = np.ascontiguousarray(brow)
    common = {
        "w1": np.ascontiguousarray(W1, dtype=np.float16),
        "w2": np.ascontiguousarray(W2, dtype=np.float16),
        "w3": np.ascontiguousarray(W3, dtype=np.float32),
        "brow": brow,
        "b2r": b2rep,
        "b3": np.ascontiguousarray(b3, dtype=np.float32).reshape(1, C),
        "p1": np.ascontiguousarray(P1, dtype=np.float32),
        "p2": np.ascontiguousarray(P2, dtype=np.float32),
        "p3": np.ascontiguousarray(P3, dtype=np.float32),
        "pb3": np.ascontiguousarray(pb3, dtype=np.float32).reshape(1, O),
    }
    in_maps = []
    for c in range(N_CORES):
        lo, hi = cuts[c], cuts[c + 1]
        n = hi - lo
        xt_c = np.zeros((E, t_sh), dtype=np.float16)
        xt_c[:, :n] = xt[:, lo:hi]
        sel_c = np.zeros((t_sh, SLOTS), dtype=np.float16)
        sel_c[:n, :] = (seg_ids[lo:hi, None] ==
                        (seg_lo[c] + np.arange(SLOTS))[None, :])
        cnt_c = np.zeros((1, SLOTS), dtype=np.float32)
        nseg = seg_hi[c] - seg_lo[c]
        cnt_c[0, :nseg] = counts[seg_lo[c]:seg_hi[c]]
        in_maps.append({
            **common,
            "xt": xt_c,
            "sel": sel_c,
            "cnt": cnt_c,
        })

    global _LAST_IN_MAPS
    _LAST_IN_MAPS = in_maps
    res = bass_utils.run_bass_kernel_spmd(nc, in_maps,
                                          core_ids=list(range(N_CORES)))
    pred = np.zeros((S, O), dtype=np.float32)
    for c in range(N_CORES):
        nseg = seg_hi[c] - seg_lo[c]
        if nseg > 0:
            pred[seg_lo[c]:seg_hi[c]] = res.results[c]["pred"][:nseg]
    return pred.reshape(bs, ai, O).astype(np.float32)


_LAST_IN_MAPS = None


# revision 14
# speedup vs baseline: 1.1733x; 1.1733x over previous
"""DeepSetPred Trainium2 kernel: token encoder MLP + segment-sum + predictor
MLP on 8 NeuronCores, zero collectives.

Sharding: the host cuts the (sorted-by-segment) token axis at segment
boundaries, so every segment belongs to exactly one core. Each shard is
padded to a common length with tokens whose one-hot selector row is all
zero. Each core computes the complete segment sums for its own contiguous
range of <=32 segments, runs the predictor on just those rows, and writes
its private slice of the output; the host concatenates.

Key algebraic restructure vs the straightforward version: the third
encoder linear (W3) commutes with the segment sum, so the per-token path
is only L1 -> tanh -> L2 -> (+b2) -> tanh -> segment-sum; W3 is applied
once to the [SLOTS, H] pooled matrix in fp32 at the tail. L2 emits
tokens-on-partitions ([tok, j]) so the segment-sum matmul consumes h2
directly (sel one-hot as the stationary operand); b2 (which then lands on
the free dim, out of reach of the ACT bias port) is added by the DVE from
a partition-replicated copy. All per-token matmuls are full K=128/M=128
fp16 with N=512 moving, which is the PE stream roofline for this network.
"""

import numpy as np

import concourse.mybir as mybir
import concourse.tile as tile
from concourse import bacc
from concourse import bass_utils
from concourse.masks import make_identity

# Problem shapes (hardcoded per contract).
T, E, H, C, O = 131072, 256, 512, 256, 32
S = 128            # num segments
N_CORES = 8
TOK = 512          # tokens per chunk
MIN_SLOTS = 32     # baseline segments-per-core capacity
F32 = mybir.dt.float32
F32R = mybir.dt.float32r
F16 = mybir.dt.float16

_CACHE = {}


def _mm(nc, out, lhsT, rhs, start, stop, skip=True):
    nc.tensor.matmul(out, lhsT, rhs,
                     start=start, stop=stop, skip_group_check=skip)


def _build_nc(t_sh, SLOTS):
    assert t_sh % 128 == 0

    nc = bacc.Bacc("TRN2", target_bir_lowering=False, debug=False,
                   num_devices=N_CORES)

    xt_d = nc.dram_tensor("xt", [E, t_sh], F16, kind="ExternalInput")
    sel_d = nc.dram_tensor("sel", [t_sh, SLOTS], F16, kind="ExternalInput")
    cnt_d = nc.dram_tensor("cnt", [1, SLOTS], F32, kind="ExternalInput")
    w1_d = nc.dram_tensor("w1", [E, H], F16, kind="ExternalInput")
    w2_d = nc.dram_tensor("w2", [H, H], F16, kind="ExternalInput")
    w3_d = nc.dram_tensor("w3", [H, C], F32R, kind="ExternalInput")
    brow_d = nc.dram_tensor("brow", [128, 12], F32, kind="ExternalInput")
    b2r_d = nc.dram_tensor("b2r", [128, H], F32, kind="ExternalInput")
    b3_d = nc.dram_tensor("b3", [1, C], F32, kind="ExternalInput")
    p1_d = nc.dram_tensor("p1", [C, H], F32R, kind="ExternalInput")
    p2_d = nc.dram_tensor("p2", [H, H], F32R, kind="ExternalInput")
    p3_d = nc.dram_tensor("p3", [H, O], F32R, kind="ExternalInput")
    pb3_d = nc.dram_tensor("pb3", [1, O], F32, kind="ExternalInput")
    out_d = nc.dram_tensor("pred", [SLOTS, O], F32, kind="ExternalOutput")

    EC = E // 128   # 2
    HC = H // 128   # 4
    CC = C // 128   # 2
    TT = TOK // 128  # 4 token sub-tiles per chunk

    with tile.TileContext(nc) as tc:
        with tc.tile_pool(name="wts", bufs=1) as wp, \
             tc.tile_pool(name="xt", bufs=3) as xtp, \
             tc.tile_pool(name="sel", bufs=4) as selp, \
             tc.tile_pool(name="h1", bufs=2) as h1p, \
             tc.tile_pool(name="z2", bufs=2) as z2p, \
             tc.tile_pool(name="h2", bufs=3) as h2p, \
             tc.tile_pool(name="small", bufs=1) as smp, \
             tc.tile_pool(name="ps", bufs=1, space="PSUM") as psp, \
             tc.tile_pool(name="psacc", bufs=1, space="PSUM") as psa:

            # warm the ACT tanh table before the scalar queue fills with DMAs
            warm_sb = smp.tile([1, 1], F32, tag="warm", name="warm")
            nc.gpsimd.memset(warm_sb[:], 0.0)
            warm_o = smp.tile([1, 1], F32, tag="warmo", name="warmo")
            nc.scalar.activation(warm_o[:], warm_sb[:],
                                 mybir.ActivationFunctionType.Tanh)

            # ---- encoder weights, needed immediately. w1/w2 are split
            # half-and-half across the scalar and vector rings so neither
            # transfer serializes behind the other; the biases ride the
            # otherwise-idle gpsimd ring as one small contiguous row. ----
            w1_ap = w1_d.ap().rearrange("(e p) (h q) -> p e h q",
                                        p=128, q=128)
            w1_t = wp.tile([128, EC, HC, 128], F16, tag="w1", name="w1t")
            nc.scalar.dma_start(w1_t[:], w1_ap)
            w1_sb = [[w1_t[:, e, h, :] for h in range(HC)] for e in range(EC)]
            w2_ap = w2_d.ap().rearrange("(k p) j -> p k j", p=128)
            w2_t = wp.tile([128, HC, H], F16, tag="w2", name="w2t")
            nc.scalar.dma_start(w2_t[:, 0:2], w2_ap[:, 0:2])
            w2_sb = [w2_t[:, k, :] for k in range(HC)]
            brow_sb = smp.tile([128, 12], F32, tag="brow", name="brow")
            nc.gpsimd.dma_start(brow_sb[:], brow_d.ap())
            b1_sb = brow_sb[:, 0:HC]
            pb1_sb = brow_sb[:, HC:2 * HC]
            pb2_sb = brow_sb[:, 2 * HC:3 * HC]
            b2r_sb = smp.tile([128, H], F32, tag="b2r", name="b2r")
            nc.gpsimd.dma_start(b2r_sb[:], b2r_d.ap())
            nc.gpsimd.dma_start(w2_t[:, 2:4], w2_ap[:, 2:4])

            # ---- PE warm-up: ~24 dummy matmuls with no DMA dependency keep
            # the tensor engine busy from t=0 so its DVFS ramp completes
            # before the first real matmul ----
            wrm = smp.tile([128, TOK], F16, tag="wrmx", name="wrmx")
            nc.gpsimd.memset(wrm[:], 0.0)
            for wi in range(24):
                pw = psp.tile([128, 1, TOK // 4], F32, tag="z", name="zw",
                              bufs=1, padded_shape=[128, 2, H])
                nc.tensor.matmul(pw[:, 0, :], wrm[:, 0:128], wrm[:, 0:TOK // 4],
                                 start=True, stop=True, skip_group_check=True)

            # ---- persistent segment-sum accumulator encH[slot, j] ----
            encH_ps = psa.tile([SLOTS, H], F32, tag="encacc", name="encacc")

            # ---- main token loop: software-pipelined three-stage skew.
            # Emit order per step i: A(i)=DMA+L1, B(i-1)=L2+bias+tanh,
            # C(i-2)=segment matmul. The 2-chunk delay on C gives the
            # DVE bias-add + ACT tanh a full PE step of slack, so the PE
            # never waits on h2. ----
            n_full = t_sh // TOK
            tail = t_sh - n_full * TOK
            chunks = [(i * TOK, TOK) for i in range(n_full)]
            if tail:
                chunks.append((n_full * TOK, tail))
            assert len(chunks) >= 3  # tail weights are emitted at ci == 2

            def stage_a(base, tok):
                xt_t = xtp.tile([128, EC, tok], F16, tag="xt", name="xt",
                                padded_shape=[128, EC, TOK])
                nc.sync.dma_start(
                    xt_t[:],
                    xt_d.ap()[:, base:base + tok]
                        .rearrange("(e p) t -> p e t", p=128))
                sel_t = selp.tile([128, tok // 128, SLOTS], F16, tag="sel",
                                  name="sel", padded_shape=[128, TT, SLOTS])
                nc.sync.dma_start(
                    sel_t[:],
                    sel_d.ap()[base:base + tok, :]
                         .rearrange("(q p) s -> p q s", p=128))
                h1_t = h1p.tile([128, HC, tok], F16, tag="h1", name="h1",
                                padded_shape=[128, HC, TOK])
                for h in range(HC):
                    ps1 = psp.tile([128, tok], F32, tag="mm", name="mm",
                                   bufs=5, padded_shape=[128, TOK])
                    for e in range(EC):
                        _mm(nc, ps1[:], w1_sb[e][h], xt_t[:, e, :],
                            start=(e == 0), stop=(e == EC - 1))
                    nc.scalar.activation(h1_t[:, h, :], ps1[:],
                                         mybir.ActivationFunctionType.Tanh,
                                         bias=b1_sb[:, h:h + 1])
                return sel_t, h1_t

            def stage_b(sel_t, h1_t, tok):
                tt = tok // 128
                z2_t = z2p.tile([128, tt, H], F32, tag="z2", name="z2",
                                padded_shape=[128, TT, H])
                h2_t = h2p.tile([128, tt, H], F16, tag="h2", name="h2",
                                padded_shape=[128, TT, H])
                for t0 in range(0, tt, 2):
                    th = min(2, tt - t0)
                    psz = psp.tile([128, th, H], F32, tag="z", name="z",
                                   bufs=1, padded_shape=[128, 2, H])
                    for t in range(th):
                        for k in range(HC):
                            _mm(nc, psz[:, t, :],
                                h1_t[:, k, (t0 + t) * 128:(t0 + t + 1) * 128],
                                w2_sb[k],
                                start=(k == 0), stop=(k == HC - 1))
                    for t in range(th):
                        nc.vector.tensor_add(z2_t[:, t0 + t, :],
                                             psz[:, t, :], b2r_sb[:])
                    nc.scalar.activation(h2_t[:, t0:t0 + th, :],
                                         z2_t[:, t0:t0 + th, :],
                                         mybir.ActivationFunctionType.Tanh)
                return sel_t, h2_t

            seg_state = {"opened": False}

            def stage_c(sel_t, h2_t, tok, is_last):
                tt = tok // 128
                for t in range(tt):
                    first = not seg_state["opened"]
                    seg_state["opened"] = True
                    last = is_last and (t == tt - 1)
                    _mm(nc, encH_ps[:], sel_t[:, t, :], h2_t[:, t, :],
                        start=first, stop=last)

            pend_b = []
            pend_c = []
            for ci, (base, tok) in enumerate(chunks):
                pend_b.append((stage_a(base, tok), tok))
                if ci == 2:
                    # tail weights: every DMA destination first gets a
                    # junk element copied from h1 (RAW on h1, then WAW for
                    # the DMA), so the scheduler cannot hoist these
                    # transfers into the startup HBM burst
                    gate = pend_b[-1][0][1]  # h1 tile of chunk 2
                    gsrc = gate[0:1, 0, 0:1]

                    def gated_dma(tile_ap, gate_dst, dram_ap):
                        nc.gpsimd.tensor_copy(gate_dst, gsrc)
                        nc.gpsimd.dma_start(tile_ap, dram_ap)

                    w3_t = wp.tile([128, HC, C], F32R, tag="w3", name="w3t")
                    gated_dma(w3_t[:], w3_t[0:1, 0, 0:1],
                              w3_d.ap().rearrange("(k p) c -> p k c", p=128))
                    p1_t = wp.tile([128, CC, HC, 128], F32R, tag="p1",
                                   name="p1t")
                    gated_dma(p1_t[:], p1_t[0:1, 0, 0, 0:1],
                              p1_d.ap().rearrange("(c p) (h q) -> p c h q",
                                                  p=128, q=128))
                    p2_t = wp.tile([128, HC, HC, 128], F32R, tag="p2",
                                   name="p2t")
                    gated_dma(p2_t[:], p2_t[0:1, 0, 0, 0:1],
                              p2_d.ap().rearrange("(k p) (h q) -> p k h q",
                                                  p=128, q=128))
                    p3_t = wp.tile([128, HC, O], F32R, tag="p3", name="p3t")
                    gated_dma(p3_t[:], p3_t[0:1, 0, 0:1],
                              p3_d.ap().rearrange("(k p) o -> p k o", p=128))
                    b3row = smp.tile([1, C], F32, tag="b3row", name="b3row")
                    gated_dma(b3row[:], b3row[0:1, 0:1], b3_d.ap())
                    pb3row = smp.tile([1, O], F32, tag="pb3row", name="pb3row")
                    gated_dma(pb3row[:], pb3row[0:1, 0:1], pb3_d.ap())
                    cntrow = smp.tile([1, SLOTS], F32, tag="cntrow",
                                      name="cntrow")
                    gated_dma(cntrow[:], cntrow[0:1, 0:1], cnt_d.ap())
                    ones1 = smp.tile([1, SLOTS], F32, tag="ones1",
                                     name="ones1")
                    nc.gpsimd.memset(ones1[:], 1.0)
                    ident = smp.tile([SLOTS, SLOTS], F32, tag="ident",
                                     name="ident")
                    make_identity(nc, ident[:])
                    w3_sb = [w3_t[:, k, :] for k in range(HC)]
                    p1_sb = [[p1_t[:, c, h, :] for h in range(HC)]
                             for c in range(CC)]
                    p2_sb = [[p2_t[:, k, h, :] for h in range(HC)]
                             for k in range(HC)]
                    p3_sb = [p3_t[:, k, :] for k in range(HC)]
                if len(pend_b) > 1:
                    ab, tokb = pend_b.pop(0)
                    pend_c.append((stage_b(*ab, tokb), tokb))
                if len(pend_c) > 1:
                    cc_, tokc = pend_c.pop(0)
                    stage_c(*cc_, tokc, is_last=False)
            while pend_b:
                ab, tokb = pend_b.pop(0)
                pend_c.append((stage_b(*ab, tokb), tokb))
            while pend_c:
                cc_, tokc = pend_c.pop(0)
                stage_c(*cc_, tokc, is_last=(len(pend_c) == 0))

            # ---- tail: pooled W3 + counts*b3, then the predictor on this
            # core's own <=SLOTS segment rows ----
            encH_sb = smp.tile([SLOTS, H], F32, tag="encHsb", name="encHsb")
            nc.vector.tensor_copy(encH_sb[:], encH_ps[:])
            encHT = smp.tile([128, HC, SLOTS], F32R, tag="encHT",
                             name="encHT")
            for k in range(HC):
                pst = psp.tile([128, SLOTS], F32, tag="mm", name="tr", bufs=5)
                nc.tensor.transpose(pst[:],
                                    encH_sb[:, k * 128:(k + 1) * 128],
                                    ident[:])
                nc.vector.tensor_copy(encHT[:, k, :], pst[:])

            # encT[c, s] = sum_k W3[k,c-slice].T @ encHT[k] + b3.T @ cnt
            encT_sb = smp.tile([128, CC, SLOTS], F32R, tag="encT",
                               name="encT")
            for c in range(CC):
                pc = psp.tile([128, SLOTS], F32, tag="mm", name="pc", bufs=5)
                nc.tensor.matmul(pc[:], b3row[:, c * 128:(c + 1) * 128],
                                 cntrow[:], start=True, stop=False,
                                 skip_group_check=True)
                for k in range(HC):
                    _mm(nc, pc[:], w3_sb[k][:, c * 128:(c + 1) * 128],
                        encHT[:, k, :], start=False, stop=(k == HC - 1))
                nc.vector.tensor_copy(encT_sb[:, c, :], pc[:])

            q1_sb = smp.tile([128, HC, SLOTS], F32R, tag="q1", name="q1")
            for h in range(HC):
                pp1 = psp.tile([128, SLOTS], F32, tag="mm", name="pp1", bufs=5)
                for c in range(CC):
                    _mm(nc, pp1[:], p1_sb[c][h], encT_sb[:, c, :],
                        start=(c == 0), stop=(c == CC - 1))
                nc.scalar.activation(q1_sb[:, h, :], pp1[:],
                                     mybir.ActivationFunctionType.Tanh,
                                     bias=pb1_sb[:, h:h + 1])
            q2_sb = smp.tile([128, HC, SLOTS], F32R, tag="q2", name="q2")
            for h in range(HC):
                pp2 = psp.tile([128, SLOTS], F32, tag="mm", name="pp2", bufs=5)
                for k in range(HC):
                    _mm(nc, pp2[:], p2_sb[k][h], q1_sb[:, k, :],
                        start=(k == 0), stop=(k == HC - 1))
                nc.scalar.activation(q2_sb[:, h, :], pp2[:],
                                     mybir.ActivationFunctionType.Tanh,
                                     bias=pb2_sb[:, h:h + 1])

            # final: pred[slot, o] = q2.T @ P3 + pb3
            ppo = psp.tile([SLOTS, O], F32, tag="mm", name="ppo", bufs=5)
            nc.tensor.matmul(ppo[:], ones1[:], pb3row[:],
                             start=True, stop=False, skip_group_check=True)
            for k in range(HC):
                _mm(nc, ppo[:], q2_sb[:, k, :], p3_sb[k],
                    start=False, stop=(k == HC - 1))
            pred_sb = smp.tile([SLOTS, O], F32, tag="pred", name="predsb")
            nc.vector.tensor_copy(pred_sb[:], ppo[:])
            nc.sync.dma_start(out_d.ap(), pred_sb[:])

    nc.compile()
    return nc


def kernel(words, seg_ids, W1, b1, W2, b2, W3, b3,
           P1, pb1, P2, pb2, P3, pb3, batch_size, alpha_iter, **_):
    words = np.asarray(words, dtype=np.float32)
    seg_ids = np.asarray(seg_ids).astype(np.int64)
    assert words.shape == (T, E), words.shape
    bs, ai = int(batch_size), int(alpha_iter)

    # --- host-side index prep: cut the sorted token axis at segment
    # boundaries so each core owns whole segments ---
    counts = np.bincount(seg_ids, minlength=S)[:S]
    starts = np.concatenate([[0], np.cumsum(counts)])   # [S+1]
    cuts = [0]
    for c in range(1, N_CORES):
        tgt = c * T // N_CORES
        j = int(np.searchsorted(starts, tgt, side="left"))
        if j > 0 and tgt - starts[j - 1] < starts[j] - tgt:
            j -= 1
        cuts.append(int(starts[j]))
    cuts.append(T)
    lens = np.diff(cuts)
    t_sh = int(np.ceil(lens.max() / 128) * 128)

    # contiguous segment range owned by each core (covers all of [0, S));
    # empty shards inherit the following shard's start so ranges stay
    # monotone and collectively exhaustive
    seg_lo = [0] * N_CORES
    for c in range(N_CORES - 1, 0, -1):
        if lens[c] > 0:
            seg_lo[c] = int(seg_ids[cuts[c]])
        else:
            seg_lo[c] = S if c == N_CORES - 1 else seg_lo[c + 1]
    seg_hi = seg_lo[1:] + [S]
    slots_needed = max(seg_hi[c] - seg_lo[c] for c in range(N_CORES))
    SLOTS = min(128, max(MIN_SLOTS, ((slots_needed + 31) // 32) * 32))
    assert slots_needed <= SLOTS, (seg_lo, seg_hi)
    assert bs * ai == S

    xt = np.ascontiguousarray(words.T.astype(np.float16))    # [E, T] fp16

    key = ("nc", t_sh, SLOTS)
    if key not in _CACHE:
        _CACHE[key] = _build_nc(t_sh, SLOTS)
    nc = _CACHE[key]

    b2rep = np.ascontiguousarray(
        np.broadcast_to(np.asarray(b2, dtype=np.float32), (128, H)))
    # [128, 12]: cols 0-3 b1, 4-7 pb1, 8-11 pb2 (per-partition bias lanes)
    brow = np.concatenate([
        np.asarray(v, dtype=np.float32).reshape(H // 128, 128).T
        for v in (b1, pb1, pb2)], axis=1)
    brow = np.ascontiguousarray(brow)
    common = {
        "w1": np.ascontiguousarray(W1, dtype=np.float16),
        "w2": np.ascontiguousarray(W2, dtype=np.float16),
        "w3": np.ascontiguousarray(W3, dtype=np.float32),
        "brow": brow,
        "b2r": b2rep,
        "b3": np.ascontiguousarray(b3, dtype=np.float32).reshape(1, C),
        "p1": np.ascontiguousarray(P1, dtype=np.float32),
        "p2": np.ascontiguousarray(P2, dtype=np.float32),
        "p3": np.ascontiguousarray(P3, dtype=np.float32),
        "pb3": np.ascontiguousarray(pb3, dtype=np.float32).reshape(1, O),
    }
    in_maps = []
    for c in range(N_CORES):
        lo, hi = cuts[c], cuts[c + 1]
        n = hi - lo
        xt_c = np.zeros((E, t_sh), dtype=np.float16)
        xt_c[:, :n] = xt[:, lo:hi]
        sel_c = np.zeros((t_sh, SLOTS), dtype=np.float16)
        sel_c[:n, :] = (seg_ids[lo:hi, None] ==
                        (seg_lo[c] + np.arange(SLOTS))[None, :])
        cnt_c = np.zeros((1, SLOTS), dtype=np.float32)
        nseg = seg_hi[c] - seg_lo[c]
        cnt_c[0, :nseg] = counts[seg_lo[c]:seg_hi[c]]
        in_maps.append({
            **common,
            "xt": xt_c,
            "sel": sel_c,
            "cnt": cnt_c,
        })

    global _LAST_IN_MAPS
    _LAST_IN_MAPS = in_maps
    res = bass_utils.run_bass_kernel_spmd(nc, in_maps,
                                          core_ids=list(range(N_CORES)))
    pred = np.zeros((S, O), dtype=np.float32)
    for c in range(N_CORES):
        nseg = seg_hi[c] - seg_lo[c]
        if nseg > 0:
            pred[seg_lo[c]:seg_hi[c]] = res.results[c]["pred"][:nseg]
    return pred.reshape(bs, ai, O).astype(np.float32)


_LAST_IN_MAPS = None


# revision 15
# speedup vs baseline: 1.1735x; 1.0002x over previous
"""DeepSetPred Trainium2 kernel: token encoder MLP + segment-sum + predictor
MLP on 8 NeuronCores, zero collectives.

Sharding: the host cuts the (sorted-by-segment) token axis at segment
boundaries, so every segment belongs to exactly one core. Each shard is
padded to a common length with tokens whose one-hot selector row is all
zero. Each core computes the complete segment sums for its own contiguous
range of <=32 segments, runs the predictor on just those rows, and writes
its private slice of the output; the host concatenates.

Key algebraic restructure vs the straightforward version: the third
encoder linear (W3) commutes with the segment sum, so the per-token path
is only L1 -> tanh -> L2 -> (+b2) -> tanh -> segment-sum; W3 is applied
once to the [SLOTS, H] pooled matrix in fp32 at the tail. L2 emits
tokens-on-partitions ([tok, j]) so the segment-sum matmul consumes h2
directly (sel one-hot as the stationary operand); b2 (which then lands on
the free dim, out of reach of the ACT bias port) is added by the DVE from
a partition-replicated copy. All per-token matmuls are full K=128/M=128
fp16 with N=512 moving, which is the PE stream roofline for this network.
"""

import numpy as np

import concourse.mybir as mybir
import concourse.tile as tile
from concourse import bacc
from concourse import bass_utils
from concourse.masks import make_identity

# Problem shapes (hardcoded per contract).
T, E, H, C, O = 131072, 256, 512, 256, 32
S = 128            # num segments
N_CORES = 8
TOK = 512          # tokens per chunk
MIN_SLOTS = 32     # baseline segments-per-core capacity
F32 = mybir.dt.float32
F32R = mybir.dt.float32r
F16 = mybir.dt.float16

_CACHE = {}


def _mm(nc, out, lhsT, rhs, start, stop, skip=True):
    nc.tensor.matmul(out, lhsT, rhs,
                     start=start, stop=stop, skip_group_check=skip)


def _build_nc(t_sh, SLOTS):
    assert t_sh % 128 == 0

    nc = bacc.Bacc("TRN2", target_bir_lowering=False, debug=False,
                   num_devices=N_CORES)

    xt_d = nc.dram_tensor("xt", [E, t_sh], F16, kind="ExternalInput")
    sel_d = nc.dram_tensor("sel", [t_sh, SLOTS], F16, kind="ExternalInput")
    cnt_d = nc.dram_tensor("cnt", [1, SLOTS], F32, kind="ExternalInput")
    w1_d = nc.dram_tensor("w1", [E, H], F16, kind="ExternalInput")
    w2_d = nc.dram_tensor("w2", [H, H], F16, kind="ExternalInput")
    w3_d = nc.dram_tensor("w3", [H, C], F32R, kind="ExternalInput")
    brow_d = nc.dram_tensor("brow", [128, 12], F32, kind="ExternalInput")
    b2r_d = nc.dram_tensor("b2r", [128, H], F32, kind="ExternalInput")
    b3_d = nc.dram_tensor("b3", [1, C], F32, kind="ExternalInput")
    p1_d = nc.dram_tensor("p1", [C, H], F32R, kind="ExternalInput")
    p2_d = nc.dram_tensor("p2", [H, H], F32R, kind="ExternalInput")
    p3_d = nc.dram_tensor("p3", [H, O], F32R, kind="ExternalInput")
    pb3_d = nc.dram_tensor("pb3", [1, O], F32, kind="ExternalInput")
    out_d = nc.dram_tensor("pred", [SLOTS, O], F32, kind="ExternalOutput")

    EC = E // 128   # 2
    HC = H // 128   # 4
    CC = C // 128   # 2
    TT = TOK // 128  # 4 token sub-tiles per chunk

    with tile.TileContext(nc) as tc:
        with tc.tile_pool(name="wts", bufs=1) as wp, \
             tc.tile_pool(name="xt", bufs=3) as xtp, \
             tc.tile_pool(name="sel", bufs=5) as selp, \
             tc.tile_pool(name="h1", bufs=2) as h1p, \
             tc.tile_pool(name="z2", bufs=2) as z2p, \
             tc.tile_pool(name="h2", bufs=4) as h2p, \
             tc.tile_pool(name="small", bufs=1) as smp, \
             tc.tile_pool(name="ps", bufs=1, space="PSUM") as psp, \
             tc.tile_pool(name="psacc", bufs=1, space="PSUM") as psa:

            # warm the ACT tanh table before the scalar queue fills with DMAs
            warm_sb = smp.tile([1, 1], F32, tag="warm", name="warm")
            nc.gpsimd.memset(warm_sb[:], 0.0)
            warm_o = smp.tile([1, 1], F32, tag="warmo", name="warmo")
            nc.scalar.activation(warm_o[:], warm_sb[:],
                                 mybir.ActivationFunctionType.Tanh)

            # ---- encoder weights, needed immediately. w1/w2 are split
            # half-and-half across the scalar and vector rings so neither
            # transfer serializes behind the other; the biases ride the
            # otherwise-idle gpsimd ring as one small contiguous row. ----
            w1_ap = w1_d.ap().rearrange("(e p) (h q) -> p e h q",
                                        p=128, q=128)
            w1_t = wp.tile([128, EC, HC, 128], F16, tag="w1", name="w1t")
            nc.scalar.dma_start(w1_t[:], w1_ap)
            w1_sb = [[w1_t[:, e, h, :] for h in range(HC)] for e in range(EC)]
            w2_ap = w2_d.ap().rearrange("(k p) j -> p k j", p=128)
            w2_t = wp.tile([128, HC, H], F16, tag="w2", name="w2t")
            nc.scalar.dma_start(w2_t[:, 0:2], w2_ap[:, 0:2])
            w2_sb = [w2_t[:, k, :] for k in range(HC)]
            brow_sb = smp.tile([128, 12], F32, tag="brow", name="brow")
            nc.gpsimd.dma_start(brow_sb[:], brow_d.ap())
            b1_sb = brow_sb[:, 0:HC]
            pb1_sb = brow_sb[:, HC:2 * HC]
            pb2_sb = brow_sb[:, 2 * HC:3 * HC]
            b2r_sb = smp.tile([128, H], F32, tag="b2r", name="b2r")
            nc.gpsimd.dma_start(b2r_sb[:], b2r_d.ap())
            nc.gpsimd.dma_start(w2_t[:, 2:4], w2_ap[:, 2:4])

            # ---- PE warm-up: ~24 dummy matmuls with no DMA dependency keep
            # the tensor engine busy from t=0 so its DVFS ramp completes
            # before the first real matmul ----
            wrm = smp.tile([128, TOK], F16, tag="wrmx", name="wrmx")
            nc.gpsimd.memset(wrm[:], 0.0)
            for wi in range(24):
                pw = psp.tile([128, 1, TOK // 4], F32, tag="z", name="zw",
                              bufs=1, padded_shape=[128, 2, H])
                nc.tensor.matmul(pw[:, 0, :], wrm[:, 0:128], wrm[:, 0:TOK // 4],
                                 start=True, stop=True, skip_group_check=True)

            # ---- persistent segment-sum accumulator encH[slot, j] ----
            encH_ps = psa.tile([SLOTS, H], F32, tag="encacc", name="encacc")

            # ---- main token loop: software-pipelined three-stage skew.
            # Emit order per step i: A(i)=DMA+L1, B(i-1)=L2+bias+tanh,
            # C(i-2)=segment matmul. The 2-chunk delay on C gives the
            # DVE bias-add + ACT tanh a full PE step of slack, so the PE
            # never waits on h2. ----
            n_full = t_sh // TOK
            tail = t_sh - n_full * TOK
            chunks = [(i * TOK, TOK) for i in range(n_full)]
            if tail:
                chunks.append((n_full * TOK, tail))
            assert len(chunks) >= 3  # tail weights are emitted at ci == 2

            def stage_a(base, tok):
                xt_t = xtp.tile([128, EC, tok], F16, tag="xt", name="xt",
                                padded_shape=[128, EC, TOK])
                nc.sync.dma_start(
                    xt_t[:],
                    xt_d.ap()[:, base:base + tok]
                        .rearrange("(e p) t -> p e t", p=128))
                sel_t = selp.tile([128, tok // 128, SLOTS], F16, tag="sel",
                                  name="sel", padded_shape=[128, TT, SLOTS])
                nc.sync.dma_start(
                    sel_t[:],
                    sel_d.ap()[base:base + tok, :]
                         .rearrange("(q p) s -> p q s", p=128))
                h1_t = h1p.tile([128, HC, tok], F16, tag="h1", name="h1",
                                padded_shape=[128, HC, TOK])
                for h in range(HC):
                    ps1 = psp.tile([128, tok], F32, tag="mm", name="mm",
                                   bufs=5, padded_shape=[128, TOK])
                    for e in range(EC):
                        _mm(nc, ps1[:], w1_sb[e][h], xt_t[:, e, :],
                            start=(e == 0), stop=(e == EC - 1))
                    nc.scalar.activation(h1_t[:, h, :], ps1[:],
                                         mybir.ActivationFunctionType.Tanh,
                                         bias=b1_sb[:, h:h + 1])
                return sel_t, h1_t

            def stage_b(sel_t, h1_t, tok):
                tt = tok // 128
                z2_t = z2p.tile([128, tt, H], F32, tag="z2", name="z2",
                                padded_shape=[128, TT, H])
                h2_t = h2p.tile([128, tt, H], F16, tag="h2", name="h2",
                                padded_shape=[128, TT, H])
                for t0 in range(0, tt, 2):
                    th = min(2, tt - t0)
                    psz = psp.tile([128, th, H], F32, tag="z", name="z",
                                   bufs=1, padded_shape=[128, 2, H])
                    for t in range(th):
                        for k in range(HC):
                            _mm(nc, psz[:, t, :],
                                h1_t[:, k, (t0 + t) * 128:(t0 + t + 1) * 128],
                                w2_sb[k],
                                start=(k == 0), stop=(k == HC - 1))
                    for t in range(th):
                        nc.vector.tensor_add(z2_t[:, t0 + t, :],
                                             psz[:, t, :], b2r_sb[:])
                    nc.scalar.activation(h2_t[:, t0:t0 + th, :],
                                         z2_t[:, t0:t0 + th, :],
                                         mybir.ActivationFunctionType.Tanh)
                return sel_t, h2_t

            seg_state = {"opened": False}

            def stage_c(sel_t, h2_t, tok, is_last):
                tt = tok // 128
                for t in range(tt):
                    first = not seg_state["opened"]
                    seg_state["opened"] = True
                    last = is_last and (t == tt - 1)
                    _mm(nc, encH_ps[:], sel_t[:, t, :], h2_t[:, t, :],
                        start=first, stop=last)

            pend_b = []
            pend_c = []
            for ci, (base, tok) in enumerate(chunks):
                pend_b.append((stage_a(base, tok), tok))
                if ci == 2:
                    # tail weights: every DMA destination first gets a
                    # junk element copied from h1 (RAW on h1, then WAW for
                    # the DMA), so the scheduler cannot hoist these
                    # transfers into the startup HBM burst
                    gate = pend_b[-1][0][1]  # h1 tile of chunk 2
                    gsrc = gate[0:1, 0, 0:1]

                    def gated_dma(tile_ap, gate_dst, dram_ap):
                        nc.gpsimd.tensor_copy(gate_dst, gsrc)
                        nc.gpsimd.dma_start(tile_ap, dram_ap)

                    w3_t = wp.tile([128, HC, C], F32R, tag="w3", name="w3t")
                    gated_dma(w3_t[:], w3_t[0:1, 0, 0:1],
                              w3_d.ap().rearrange("(k p) c -> p k c", p=128))
                    p1_t = wp.tile([128, CC, HC, 128], F32R, tag="p1",
                                   name="p1t")
                    gated_dma(p1_t[:], p1_t[0:1, 0, 0, 0:1],
                              p1_d.ap().rearrange("(c p) (h q) -> p c h q",
                                                  p=128, q=128))
                    p2_t = wp.tile([128, HC, HC, 128], F32R, tag="p2",
                                   name="p2t")
                    gated_dma(p2_t[:], p2_t[0:1, 0, 0, 0:1],
                              p2_d.ap().rearrange("(k p) (h q) -> p k h q",
                                                  p=128, q=128))
                    p3_t = wp.tile([128, HC, O], F32R, tag="p3", name="p3t")
                    gated_dma(p3_t[:], p3_t[0:1, 0, 0:1],
                              p3_d.ap().rearrange("(k p) o -> p k o", p=128))
                    b3row = smp.tile([1, C], F32, tag="b3row", name="b3row")
                    gated_dma(b3row[:], b3row[0:1, 0:1], b3_d.ap())
                    pb3row = smp.tile([1, O], F32, tag="pb3row", name="pb3row")
                    gated_dma(pb3row[:], pb3row[0:1, 0:1], pb3_d.ap())
                    cntrow = smp.tile([1, SLOTS], F32, tag="cntrow",
                                      name="cntrow")
                    gated_dma(cntrow[:], cntrow[0:1, 0:1], cnt_d.ap())
                    ones1 = smp.tile([1, SLOTS], F32, tag="ones1",
                                     name="ones1")
                    nc.gpsimd.memset(ones1[:], 1.0)
                    ident = smp.tile([SLOTS, SLOTS], F32, tag="ident",
                                     name="ident")
                    make_identity(nc, ident[:])
                    w3_sb = [w3_t[:, k, :] for k in range(HC)]
                    p1_sb = [[p1_t[:, c, h, :] for h in range(HC)]
                             for c in range(CC)]
                    p2_sb = [[p2_t[:, k, h, :] for h in range(HC)]
                             for k in range(HC)]
                    p3_sb = [p3_t[:, k, :] for k in range(HC)]
                if len(pend_b) > 1:
                    ab, tokb = pend_b.pop(0)
                    pend_c.append((stage_b(*ab, tokb), tokb))
                if len(pend_c) > 2:
                    cc_, tokc = pend_c.pop(0)
                    stage_c(*cc_, tokc, is_last=False)
            while pend_b:
                ab, tokb = pend_b.pop(0)
                pend_c.append((stage_b(*ab, tokb), tokb))
            while pend_c:
                cc_, tokc = pend_c.pop(0)
                stage_c(*cc_, tokc, is_last=(len(pend_c) == 0))

            # ---- tail: pooled W3 + counts*b3, then the predictor on this
            # core's own <=SLOTS segment rows ----
            encH_sb = smp.tile([SLOTS, H], F32, tag="encHsb", name="encHsb")
            nc.vector.tensor_copy(encH_sb[:], encH_ps[:])
            encHT = smp.tile([128, HC, SLOTS], F32R, tag="encHT",
                             name="encHT")
            for k in range(HC):
                pst = psp.tile([128, SLOTS], F32, tag="mm", name="tr", bufs=5)
                nc.tensor.transpose(pst[:],
                                    encH_sb[:, k * 128:(k + 1) * 128],
                                    ident[:])
                nc.vector.tensor_copy(encHT[:, k, :], pst[:])

            # encT[c, s] = sum_k W3[k,c-slice].T @ encHT[k] + b3.T @ cnt
            encT_sb = smp.tile([128, CC, SLOTS], F32R, tag="encT",
                               name="encT")
            for c in range(CC):
                pc = psp.tile([128, SLOTS], F32, tag="mm", name="pc", bufs=5)
                nc.tensor.matmul(pc[:], b3row[:, c * 128:(c + 1) * 128],
                                 cntrow[:], start=True, stop=False,
                                 skip_group_check=True)
                for k in range(HC):
                    _mm(nc, pc[:], w3_sb[k][:, c * 128:(c + 1) * 128],
                        encHT[:, k, :], start=False, stop=(k == HC - 1))
                nc.vector.tensor_copy(encT_sb[:, c, :], pc[:])

            q1_sb = smp.tile([128, HC, SLOTS], F32R, tag="q1", name="q1")
            for h in range(HC):
                pp1 = psp.tile([128, SLOTS], F32, tag="mm", name="pp1", bufs=5)
                for c in range(CC):
                    _mm(nc, pp1[:], p1_sb[c][h], encT_sb[:, c, :],
                        start=(c == 0), stop=(c == CC - 1))
                nc.scalar.activation(q1_sb[:, h, :], pp1[:],
                                     mybir.ActivationFunctionType.Tanh,
                                     bias=pb1_sb[:, h:h + 1])
            q2_sb = smp.tile([128, HC, SLOTS], F32R, tag="q2", name="q2")
            for h in range(HC):
                pp2 = psp.tile([128, SLOTS], F32, tag="mm", name="pp2", bufs=5)
                for k in range(HC):
                    _mm(nc, pp2[:], p2_sb[k][h], q1_sb[:, k, :],
                        start=(k == 0), stop=(k == HC - 1))
                nc.scalar.activation(q2_sb[:, h, :], pp2[:],
                                     mybir.ActivationFunctionType.Tanh,
                                     bias=pb2_sb[:, h:h + 1])

            # final: pred[slot, o] = q2.T @ P3 + pb3
            ppo = psp.tile([SLOTS, O], F32, tag="mm", name="ppo", bufs=5)
            nc.tensor.matmul(ppo[:], ones1[:], pb3row[:],
                             start=True, stop=False, skip_group_check=True)
            for k in range(HC):
                _mm(nc, ppo[:], q2_sb[:, k, :], p3_sb[k],
                    start=False, stop=(k == HC - 1))
            pred_sb = smp.tile([SLOTS, O], F32, tag="pred", name="predsb")
            nc.vector.tensor_copy(pred_sb[:], ppo[:])
            nc.sync.dma_start(out_d.ap(), pred_sb[:])

    nc.compile()
    return nc


def kernel(words, seg_ids, W1, b1, W2, b2, W3, b3,
           P1, pb1, P2, pb2, P3, pb3, batch_size, alpha_iter, **_):
    words = np.asarray(words, dtype=np.float32)
    seg_ids = np.asarray(seg_ids).astype(np.int64)
    assert words.shape == (T, E), words.shape
    bs, ai = int(batch_size), int(alpha_iter)

    # --- host-side index prep: cut the sorted token axis at segment
    # boundaries so each core owns whole segments ---
    counts = np.bincount(seg_ids, minlength=S)[:S]
    starts = np.concatenate([[0], np.cumsum(counts)])   # [S+1]
    cuts = [0]
    for c in range(1, N_CORES):
        tgt = c * T // N_CORES
        j = int(np.searchsorted(starts, tgt, side="left"))
        if j > 0 and tgt - starts[j - 1] < starts[j] - tgt:
            j -= 1
        cuts.append(int(starts[j]))
    cuts.append(T)
    lens = np.diff(cuts)
    t_sh = int(np.ceil(lens.max() / 128) * 128)

    # contiguous segment range owned by each core (covers all of [0, S));
    # empty shards inherit the following shard's start so ranges stay
    # monotone and collectively exhaustive
    seg_lo = [0] * N_CORES
    for c in range(N_CORES - 1, 0, -1):
        if lens[c] > 0:
            seg_lo[c] = int(seg_ids[cuts[c]])
        else:
            seg_lo[c] = S if c == N_CORES - 1 else seg_lo[c + 1]
    seg_hi = seg_lo[1:] + [S]
    slots_needed = max(seg_hi[c] - seg_lo[c] for c in range(N_CORES))
    SLOTS = min(128, max(MIN_SLOTS, ((slots_needed + 31) // 32) * 32))
    assert slots_needed <= SLOTS, (seg_lo, seg_hi)
    assert bs * ai == S

    xt = np.ascontiguousarray(words.T.astype(np.float16))    # [E, T] fp16

    key = ("nc", t_sh, SLOTS)
    if key not in _CACHE:
        _CACHE[key] = _build_nc(t_sh, SLOTS)
    nc = _CACHE[key]

    b2rep = np.ascontiguousarray(
        np.broadcast_to(np.asarray(b2, dtype=np.float32), (128, H)))
    # [128, 12]: cols 0-3 b1, 4-7 pb1, 8-11 pb2 (per-partition bias lanes)
    brow = np.concatenate([
        np.asarray(v, dtype=np.float32).reshape(H // 128, 128).T
        for v in (b1, pb1, pb2)], axis=1)
    brow = np.ascontiguousarray(brow)
    common = {
        "w1": np.ascontiguousarray(W1, dtype=np.float16),
        "w2": np.ascontiguousarray(W2, dtype=np.float16),
        "w3": np.ascontiguousarray(W3, dtype=np.float32),
        "brow": brow,
        "b2r": b2rep,
        "b3": np.ascontiguousarray(b3, dtype=np.float32).reshape(1, C),
        "p1": np.ascontiguousarray(P1, dtype=np.float32),
        "p2": np.ascontiguousarray(P2, dtype=np.float32),
        "p3": np.ascontiguousarray(P3, dtype=np.float32),
        "pb3": np.ascontiguousarray(pb3, dtype=np.float32).reshape(1, O),
    }
    in_maps = []
    for c in range(N_CORES):
        lo, hi = cuts[c], cuts[c + 1]
        n = hi - lo
        xt_c = np.zeros((E, t_sh), dtype=np.float16)
        xt_c[:, :n] = xt[:, lo:hi]
        sel_c = np.zeros((t_sh, SLOTS), dtype=np.float16)
        sel_c[:n, :] = (seg_ids[lo:hi, None] ==
                        (seg_lo[c] + np.arange(SLOTS))[None, :])
        cnt_c = np.zeros((1, SLOTS), dtype=np.float32)
        nseg = seg_hi[c] - seg_lo[c]
        cnt_c[0, :nseg] = counts[seg_lo[c]:seg_hi[c]]
        in_maps.append({
            **common,
            "xt": xt_c,
            "sel": sel_c,
            "cnt": cnt_c,
        })

    global _LAST_IN_MAPS
    _LAST_IN_MAPS = in_maps
    res = bass_utils.run_bass_kernel_spmd(nc, in_maps,
                                          core_ids=list(range(N_CORES)))
    pred = np.zeros((S, O), dtype=np.float32)
    for c in range(N_CORES):
        nseg = seg_hi[c] - seg_lo[c]
        if nseg > 0:
            pred[seg_lo[c]:seg_hi[c]] = res.results[c]["pred"][:nseg]
    return pred.reshape(bs, ai, O).astype(np.float32)


_LAST_IN_MAPS = None
